# revision 5
# baseline (speedup 1.0000x reference)
"""nn_HWTConv2D Trainium2 kernel, v2 — wire-optimized pipeline.

y = x + iHaar2d( sum_p SoftThresh( conv1x1_p( Haar2d(x) * v_p ), tau_p ) )

The axon tunnel (~45MB/s, shared half-duplex) dominates wall time, so v3:
  * uploads x as 5-bit codes packed 8/5bytes with per-row fp32 scales
    (42MB), split into 2 chunks x 4 channel-groups, each packed +
    device_put async so packing overlaps the wire;
  * keeps v/conv_w/tau + Haar matrices resident on device across calls
    (re-uploaded only if their bytes change);
  * downloads corr as int4 codes packed 2/byte with per-16-element fp8
    block scales (37.7MB total vs 67MB), quantized on device with
    magic-number round-to-nearest (bit-exact vs numpy, verified);
  * unpacks on host with a 256-entry float64-pair LUT (one gather) and
    fuses the residual add, overlapping the remaining downloads at
    8-channel granularity (16 download+unpack units per call).

Device program (per core, one batch per exec):
  S1/S2  forward Haar, data-stationary matmuls (fp8 data x bf16 HM^T).
  conv   channel mix via DMA-gathered [c|pix] tiles, one W^T per pod.
  thresh f5 = t - clip(t, +-tau) as min/mult ops.
  I_h/I_w inverse Haar; each [128,256] fp32 PSUM plane-half is then
         block-quantized: absmax/16 -> fp8 scale -> reciprocal ->
         q = RN(v/s)+8 in [0,15] -> pack hi*16+lo -> uint8 DMA.
"""

import threading

import numpy as np
import ml_dtypes

B, C, H, W, P = 16, 64, 256, 256, 2
NCORES = 8
NCHUNK = 2            # execs per call; chunk k = batches k*8 .. k*8+7
NSPLIT = 4            # xs channel-group splits per chunk
CSP = C // NSPLIT     # channels per split
BF16 = ml_dtypes.bfloat16
FP8 = ml_dtypes.float8_e4m3
NORM = float(1.0 / np.sqrt(2.0))
MAGIC = 8388608.0     # 2^23: (v + MAGIC) - MAGIC == round-to-nearest(v), v >= 0.25
MAGIC2 = 12582912.0   # 1.5*2^23: same trick valid for v >= -0.5 (unpack floors)
QSCALE = 1.0 / 7.49   # block absmax -> quant scale
SMIN = 2.0 ** -9      # fp8 subnormal floor for all-zero blocks
ISCALE = 1.0 / 15.49  # input row absmax -> 5-bit scale

_lock = threading.Lock()
_state: dict = {}


def _haar_matrix(n):
    # Orthonormal multilevel 1D Haar matrix: haar1d_fwd(x) == HM @ x.
    m = int(np.log2(n))
    hm = np.eye(n, dtype=np.float64)
    length = n
    for _ in range(m):
        lvl = np.eye(n, dtype=np.float64)
        half = length // 2
        blk = np.zeros((length, length), dtype=np.float64)
        for i in range(half):
            blk[i, 2 * i] = NORM
            blk[i, 2 * i + 1] = NORM
            blk[half + i, 2 * i] = NORM
            blk[half + i, 2 * i + 1] = -NORM
        lvl[:length, :length] = blk
        hm = lvl @ hm
        length //= 2
    return hm.astype(np.float32)


def _build_nc():
    import concourse.bacc as bacc
    import concourse.tile as tile
    from concourse import mybir
    from contextlib import ExitStack

    dt = mybir.dt
    alu = mybir.AluOpType
    nc = bacc.Bacc("TRN2", target_bir_lowering=False, debug=False)

    xps = [nc.dram_tensor(f"xp{s}", [CSP, 2, 128, 160], dt.uint8,
                          kind="ExternalInput") for s in range(NSPLIT)]
    scs = [nc.dram_tensor(f"sc{s}", [128, CSP, 2], dt.float32,
                          kind="ExternalInput") for s in range(NSPLIT)]
    hmt_s = nc.dram_tensor("hmt_s", [128, 2, 256], dt.bfloat16, kind="ExternalInput")
    hm_s = nc.dram_tensor("hm_s", [128, 2, 256], dt.bfloat16, kind="ExternalInput")
    wdup = nc.dram_tensor("wdup", [64, P, 64], dt.bfloat16, kind="ExternalInput")
    v_s = nc.dram_tensor("v_s", [128, P, 2, 256], dt.bfloat16, kind="ExternalInput")
    tau_s = nc.dram_tensor("tau_s", [128, P, 2, 256], dt.bfloat16, kind="ExternalInput")
    pks = [nc.dram_tensor(f"pk{h}", [C // 8, 2, 128, 128], dt.uint8,
                          kind="ExternalOutput") for h in range(8)]
    s8_out = nc.dram_tensor("s8", [C, 2, 128, 16], dt.float8e4, kind="ExternalOutput")

    with ExitStack() as ctx:
        tc = ctx.enter_context(tile.TileContext(nc))
        pc = ctx.enter_context(tc.tile_pool(name="consts", bufs=1))
        pw = ctx.enter_context(tc.tile_pool(name="work", bufs=1))
        pps = ctx.enter_context(tc.tile_pool(name="ps", bufs=4, space="PSUM"))
        pps3 = ctx.enter_context(tc.tile_pool(name="ps3", bufs=4, space="PSUM"))

        hmt_sb = pc.tile_from(hmt_s[:])
        hm_sb = pc.tile_from(hm_s[:])
        wdup_sb = pc.tile_from(wdup[:])
        v_sb = pc.tile_from(v_s[:])
        tau_sb = pc.tile_from(tau_s[:])

        f1 = pw.tile([128, 2, C, 256], dt.bfloat16, tag="f1")
        q = pw.tile([128, 2, C, 256], dt.bfloat16, tag="q")
        q2 = pw.tile([128, C, 256], dt.bfloat16, tag="q2")
        xcs = [pw.tile([128, 2, 256], dt.bfloat16, tag=f"xc{i}", name=f"xc{i}") for i in range(3)]
        # 6-bit input unpack tiles
        sc_sb = [pc.tile_from(scs[s][:], name=f"sc_sb{s}") for s in range(NSPLIT)]
        bus = [pw.tile([128, 2, 160], dt.uint8, tag=f"bu{i}", name=f"bu{i}") for i in range(2)]
        bfs = [pw.tile([128, 2, 160], dt.float32, tag=f"bfc{i}", name=f"bfc{i}") for i in range(2)]
        ufl = pw.tile([128, 32], dt.float32, tag="ufl")
        ut1 = pw.tile([128, 32], dt.float32, tag="ut1")
        ut3 = pw.tile([128, 32], dt.float32, tag="ut3")
        ut4 = pw.tile([128, 32], dt.float32, tag="ut4")
        ut6 = pw.tile([128, 32], dt.float32, tag="ut6")
        ur0 = pw.tile([128, 32], dt.float32, tag="ur0")
        ur1 = pw.tile([128, 32], dt.float32, tag="ur1")
        ur2 = pw.tile([128, 32], dt.float32, tag="ur2")
        ur3 = pw.tile([128, 32], dt.float32, tag="ur3")
        uc3h = pw.tile([128, 32], dt.float32, tag="uc3h")
        uc6h = pw.tile([128, 32], dt.float32, tag="uc6h")
        uc8s = [pw.tile([128, 32, 8], dt.float32, tag=f"uc8{i}", name=f"uc8{i}") for i in range(2)]
        r1s_ = [pw.tile([128, 256], dt.bfloat16, tag=f"r1{i}", name=f"r1{i}") for i in range(4)]
        ftcs = [pw.tile([64, 8 * 256], dt.bfloat16, tag=f"ftc{i}", name=f"ftc{i}") for i in range(2)]
        sgs = [pw.tile([64, 8 * 256], dt.bfloat16, tag=f"sg{i}", name=f"sg{i}") for i in range(2)]
        tts = [pw.tile([128, 8, 256], dt.bfloat16, tag="tt0", name="tt0")]
        mms = [pw.tile([128, 8, 256], dt.bfloat16, tag="mm0", name="mm0")]
        t2s = [pw.tile([128, 2, 256], dt.bfloat16, tag=f"t2{i}", name=f"t2{i}") for i in range(2)]
        # quantization work tiles (rotated)
        ams = [pw.tile([128, 16], dt.float32, tag=f"am{i}", name=f"am{i}") for i in range(2)]
        s8s = [pw.tile([128, 16], dt.float8e4, tag=f"s8{i}", name=f"s8{i}") for i in range(2)]
        sbs = [pw.tile([128, 16], dt.float32, tag=f"sb{i}", name=f"sb{i}") for i in range(2)]
        qts = [pw.tile([128, 16, 16], dt.float32, tag=f"qt{i}", name=f"qt{i}") for i in range(2)]
        pkts = [pw.tile([128, 128], dt.uint8, tag=f"pkt{i}", name=f"pkt{i}") for i in range(2)]

        # ---- forward transform: per-plane, 6-bit unpack + fused S1+S2 ----
        for c in range(C):
            xc = xcs[c % 3]
            bu = bus[c % 2]
            bf = bfs[c % 2]
            c8 = uc8s[c % 2]
            xsrc = xps[c // CSP]
            for hh in range(2):
                nc.gpsimd.dma_start(bu[:, hh, :], xsrc[c % CSP, hh])
            nc.any.tensor_copy(bf[:], bu[:])          # u8 -> fp32 exact
            stt = nc.vector.scalar_tensor_tensor

            def fl(dst, srcap, inv, d):
                # dst = floor(srcap * inv), via exact-offset + magic RN
                nc.vector.tensor_scalar(ufl[:], srcap, inv, -d, alu.mult, alu.add)
                nc.vector.tensor_scalar(dst, ufl[:], MAGIC2, -MAGIC2, alu.add, alu.add)

            for hh in range(2):
                bv = bf[:, hh, :].rearrange("p (g five) -> p g five", five=5)
                B0, B1, B2, B3, B4 = (bv[:, :, i] for i in range(5))
                fl(c8[:, :, 0], B0, 0.125, 0.4375)                        # c0
                stt(ur0[:], c8[:, :, 0], -8.0, B0, alu.mult, alu.add)
                fl(ut1[:], B1, 1.0 / 64.0, 0.4921875)                     # c1&3
                stt(c8[:, :, 1], ur0[:], 4.0, ut1[:], alu.mult, alu.add)  # c1
                stt(ur1[:], ut1[:], -64.0, B1, alu.mult, alu.add)
                fl(c8[:, :, 2], ur1[:], 0.5, 0.25)                        # c2
                stt(uc3h[:], c8[:, :, 2], -2.0, ur1[:], alu.mult, alu.add)
                fl(ut3[:], B2, 1.0 / 16.0, 0.46875)                       # c3&15
                stt(c8[:, :, 3], uc3h[:], 16.0, ut3[:], alu.mult, alu.add)
                stt(ur2[:], ut3[:], -16.0, B2, alu.mult, alu.add)
                fl(ut4[:], B3, 1.0 / 128.0, 0.498046875)                  # c4&1
                stt(c8[:, :, 4], ur2[:], 2.0, ut4[:], alu.mult, alu.add)
                stt(ur3[:], ut4[:], -128.0, B3, alu.mult, alu.add)
                fl(c8[:, :, 5], ur3[:], 0.25, 0.375)                      # c5
                stt(uc6h[:], c8[:, :, 5], -4.0, ur3[:], alu.mult, alu.add)
                fl(ut6[:], B4, 1.0 / 32.0, 0.484375)                      # c6&7
                stt(c8[:, :, 6], uc6h[:], 8.0, ut6[:], alu.mult, alu.add)
                stt(c8[:, :, 7], ut6[:], -32.0, B4, alu.mult, alu.add)    # c7
                # dequant: xc = (codes - 16) * scale_row
                cvf = c8[:].rearrange("p g e -> p (g e)")
                scb = sc_sb[c // CSP][:, c % CSP, hh].unsqueeze(1).broadcast_to([128, 256])
                stt(xc[:, hh, :], cvf, -16.0, scb, alu.add, alu.mult)
            r1s = []
            for wh in range(2):
                ps1 = pps.tile([128, 256], dt.float32, tag="ps")
                nc.tensor.matmul(ps1[:], xc[:, 0, wh * 128:(wh + 1) * 128],
                                 hmt_sb[:, 0, :], start=True, stop=False)
                nc.tensor.matmul(ps1[:], xc[:, 1, wh * 128:(wh + 1) * 128],
                                 hmt_sb[:, 1, :], start=False, stop=True)
                r1 = r1s_[(2 * c + wh) % 4]
                nc.any.tensor_copy(r1[:], ps1[:])
                r1s.append(r1)
            for hph in range(2):
                ps2 = pps.tile([128, 256], dt.float32, tag="ps")
                nc.tensor.matmul(ps2[:], r1s[0][:, hph * 128:(hph + 1) * 128],
                                 hmt_sb[:, 0, :], start=True, stop=False)
                nc.tensor.matmul(ps2[:], r1s[1][:, hph * 128:(hph + 1) * 128],
                                 hmt_sb[:, 1, :], start=False, stop=True)
                nc.any.tensor_copy(f1[:, hph, c, :], ps2[:])

        # ---- conv (channel mix) + soft-threshold ----
        for hph in range(2):
            for chk in range(16):
                ftc = ftcs[chk % 2]
                ftv = ftc[:].rearrange("c (hl w) -> c hl w", hl=8)
                for hl in range(8):
                    row = chk * 8 + hl
                    nc.gpsimd.dma_start(ftv[:, hl, :], f1[row:row + 1, hph, :, :])
                for pod in range(P):
                    sg = sgs[pod]
                    for q4 in range(4):
                        ps3 = pps3.tile([64, 512], dt.float32, tag="ps3")
                        nc.tensor.matmul(ps3[:], wdup_sb[:, pod, :],
                                         ftc[:, q4 * 512:(q4 + 1) * 512],
                                         start=True, stop=True)
                        nc.any.tensor_copy(sg[:, q4 * 512:(q4 + 1) * 512], ps3[:])
                    dst = q if pod == 0 else q2
                    sgv = sg[:].rearrange("o (hl w) -> o hl w", hl=8)
                    for hl in range(8):
                        row = chk * 8 + hl
                        drow = (dst[row:row + 1, hph, :, :] if pod == 0
                                else dst[row:row + 1, :, :])
                        nc.gpsimd.dma_start(drow, sgv[:, hl, :])
            for ch2 in range(8):
                osl = slice(ch2 * 8, (ch2 + 1) * 8)
                qs = q[:, hph, osl, :]
                q2s = q2[:, osl, :]
                t = tts[0]
                m = mms[0]
                vb0 = v_sb[:, 0, hph, :].unsqueeze(1).broadcast_to([128, 8, 256])
                tb0 = tau_sb[:, 0, hph, :].unsqueeze(1).broadcast_to([128, 8, 256])
                vb1 = v_sb[:, 1, hph, :].unsqueeze(1).broadcast_to([128, 8, 256])
                tb1 = tau_sb[:, 1, hph, :].unsqueeze(1).broadcast_to([128, 8, 256])
                # pod0, in place: q <- t + min(-min(t,tau), tau) = t - clip(t)
                nc.vector.tensor_tensor(t[:], qs, vb0, alu.mult)
                nc.vector.tensor_tensor(m[:], t[:], tb0, alu.min)
                nc.vector.scalar_tensor_tensor(m[:], m[:], -1.0, tb0, alu.mult, alu.min)
                nc.vector.tensor_tensor(qs, t[:], m[:], alu.add)
                # pod1, accumulate into q
                nc.vector.tensor_tensor(t[:], q2s, vb1, alu.mult)
                nc.vector.tensor_tensor(m[:], t[:], tb1, alu.min)
                nc.vector.scalar_tensor_tensor(m[:], m[:], -1.0, tb1, alu.mult, alu.min)
                nc.vector.tensor_tensor(t[:], t[:], m[:], alu.add)
                nc.vector.tensor_tensor(qs, qs, t[:], alu.add)

        # ---- inverse transform + int4-blk16 quantization, per plane ----
        for o in range(C):
            t2 = t2s[o % 2]
            for wph in range(2):
                ps = pps.tile([128, 256], dt.float32, tag="ps")
                nc.tensor.matmul(ps[:], q[:, 0, o, wph * 128:(wph + 1) * 128],
                                 hm_sb[:, 0, :], start=True, stop=False)
                nc.tensor.matmul(ps[:], q[:, 1, o, wph * 128:(wph + 1) * 128],
                                 hm_sb[:, 1, :], start=False, stop=True)
                nc.any.tensor_copy(t2[:, wph, :], ps[:])
            for hh in range(2):
                ps = pps.tile([128, 256], dt.float32, tag="ps")
                nc.tensor.matmul(ps[:], t2[:, 0, hh * 128:(hh + 1) * 128],
                                 hm_sb[:, 0, :], start=True, stop=False)
                nc.tensor.matmul(ps[:], t2[:, 1, hh * 128:(hh + 1) * 128],
                                 hm_sb[:, 1, :], start=False, stop=True)
                k = (2 * o + hh) % 2
                am, s8, sb, qt, pkt = ams[k], s8s[k], sbs[k], qts[k], pkts[k]
                cv = ps[:].rearrange("p (b e) -> p b e", b=16)
                nc.vector.tensor_reduce(am[:], cv, axis=mybir.AxisListType.X,
                                        op=alu.max, apply_absolute_value=True)
                nc.vector.tensor_scalar_mul(am[:], am[:], QSCALE)
                nc.vector.tensor_scalar_max(am[:], am[:], SMIN)
                nc.any.tensor_copy(s8[:], am[:])     # fp32 -> fp8 (wire scale)
                nc.any.tensor_copy(sb[:], s8[:])     # fp8 -> fp32 (consistent)
                nc.vector.reciprocal(sb[:], sb[:])
                sb_b = sb[:].unsqueeze(2).broadcast_to([128, 16, 16])
                nc.vector.tensor_tensor(qt[:], cv, sb_b, alu.mult)
                nc.vector.tensor_scalar_add(qt[:], qt[:], 8.0 + MAGIC)
                nc.vector.tensor_scalar_add(qt[:], qt[:], -MAGIC)
                nc.vector.tensor_scalar_min(qt[:], qt[:], 15.0)
                qv = qt[:].rearrange("p b e -> p (b e)").rearrange(
                    "p (k two) -> p k two", two=2)
                nc.vector.scalar_tensor_tensor(pkt[:], qv[:, :, 0], 16.0,
                                               qv[:, :, 1], alu.mult, alu.add)
                nc.gpsimd.dma_start(pks[o // (C // 8)][o % (C // 8), hh], pkt[:])
                nc.gpsimd.dma_start(s8_out[o, hh], s8[:])

    nc.compile()
    nc.finalize()
    return nc


def _prep_consts(v, conv_w, tau):
    hm = _haar_matrix(H)
    hmt = np.ascontiguousarray(hm.T)
    hmt_s = np.ascontiguousarray(hmt.reshape(2, 128, 256).transpose(1, 0, 2)).astype(BF16)
    hm_s = np.ascontiguousarray(hm.reshape(2, 128, 256).transpose(1, 0, 2)).astype(BF16)
    wdup = np.ascontiguousarray(conv_w.transpose(2, 0, 1)).astype(BF16)  # [c, pod, o]
    v_s = np.ascontiguousarray(
        v.reshape(P, 2, 128, 256).transpose(2, 0, 1, 3)).astype(BF16)
    tau_s = np.ascontiguousarray(
        tau.reshape(P, 2, 128, 256).transpose(2, 0, 1, 3)).astype(BF16)
    return {"hmt_s": hmt_s, "hm_s": hm_s, "wdup": wdup, "v_s": v_s, "tau_s": tau_s}


def _build_fast(nc, mesh):
    """jit(shard_map) executing the bass NEFF on 8 cores (bass2jax custom
    call), with device-resident zero output buffers."""
    import jax
    from jax.sharding import PartitionSpec, NamedSharding
    try:
        from jax.experimental.shard_map import shard_map
    except ImportError:
        from jax.shard_map import shard_map
    from concourse import bass2jax, mybir

    bass2jax.install_neuronx_cc_hook()

    pname = nc.partition_id_tensor.name if nc.partition_id_tensor else None
    in_names, out_names, out_avals = [], [], []
    for alloc in nc.m.functions[0].allocations:
        if not isinstance(alloc, mybir.MemoryLocationSet):
            continue
        name = alloc.memorylocations[0].name
        if alloc.kind == "ExternalInput":
            if name != pname:
                in_names.append(name)
        elif alloc.kind == "ExternalOutput":
            out_names.append(name)
            out_avals.append(jax.core.ShapedArray(
                tuple(alloc.tensor_shape), mybir.dt.np(alloc.dtype)))

    bind_names = list(in_names) + list(out_names)
    if pname is not None:
        bind_names.append(pname)

    def _body(*args):
        operands = list(args)
        if pname is not None:
            operands.append(bass2jax.partition_id_tensor())
        outs = bass2jax._bass_exec_p.bind(
            *operands,
            out_avals=tuple(out_avals),
            in_names=tuple(bind_names),
            out_names=tuple(out_names),
            lowering_input_output_aliases=(),
            sim_require_finite=True,
            sim_require_nnan=True,
            nc=nc,
        )
        return tuple(outs)

    n_args = len(in_names) + len(out_avals)
    fast = jax.jit(shard_map(
        _body, mesh=mesh,
        in_specs=(PartitionSpec("core"),) * n_args,
        out_specs=(PartitionSpec("core"),) * len(out_names),
        check_rep=False))
    sh = NamedSharding(mesh, PartitionSpec("core"))
    zeros_dev = [
        jax.device_put(
            np.zeros((NCORES * a.shape[0], *a.shape[1:]), a.dtype), sh)
        for a in out_avals
    ]
    for z in zeros_dev:
        z.block_until_ready()
    return fast, in_names, out_names, zeros_dev


def _make_luts():
    # packed byte -> (hi_code-8, lo_code-8) as adjacent fp32 in one f64 slot
    idx = np.arange(256)
    lut64 = np.empty(256, dtype="<f8")
    lv = lut64.view("<f4").reshape(256, 2)
    lv[:, 0] = (idx >> 4) - 8.0
    lv[:, 1] = (idx & 15) - 8.0
    # fp8 byte -> fp32 scale
    lut_fp8 = np.arange(256, dtype=np.uint8).view(FP8).astype(np.float32)
    return lut64, lut_fp8


def _ensure_built():
    import jax
    from jax.sharding import Mesh
    if "fast" in _state:
        return
    nc = _build_nc()
    mesh = Mesh(np.asarray(jax.devices()[:NCORES]), ("core",))
    _state["mesh"] = mesh
    _state["fast"] = _build_fast(nc, mesh)
    _state["lut64"], _state["lut_fp8"] = _make_luts()
    _state["xh"] = [[np.empty((NCORES * CSP, 2, 128, 160), dtype=np.uint8)
                     for _ in range(NSPLIT)] for _ in range(NCHUNK)]
    _state["sch"] = [[np.empty((NCORES * 128, CSP, 2), dtype=np.float32)
                      for _ in range(NSPLIT)] for _ in range(NCHUNK)]
    _state["qtmp"] = np.empty((NCORES, CSP, 2, 128, 256), dtype=np.float32)
    _state["ctmp"] = np.empty((NCORES, CSP, 2, 128, 256), dtype=np.uint8)
    _state["ybuf"] = np.empty((B, C, H, W), dtype=np.float32)


def _ensure_consts(v, conv_w, tau):
    import jax
    from jax.sharding import PartitionSpec, NamedSharding
    key = (v.tobytes(), conv_w.tobytes(), tau.tobytes())
    cached = _state.get("consts_key")
    if cached is not None and cached == key:
        return
    consts = _prep_consts(v, conv_w, tau)
    sh = NamedSharding(_state["mesh"], PartitionSpec("core"))
    dev = {}
    for n, a in consts.items():
        g = np.broadcast_to(a[None], (NCORES,) + a.shape).reshape(
            (NCORES * a.shape[0],) + a.shape[1:])
        dev[n] = jax.device_put(np.ascontiguousarray(g), sh)
    for d in dev.values():
        d.block_until_ready()
    _state["consts_dev"] = dev
    _state["consts_key"] = key


def _pack_chunk_split(x, k, s):
    """6-bit rowscale pack of batches k*8.., channels s*CSP.. ->
    (packed [8*CSP,2,128,192] u8, scales [8*128,CSP,2] f32)."""
    pkb = _state["xh"][k][s]
    scb = _state["sch"][k][s]
    tmp = _state["qtmp"]
    cb = _state["ctmp"]
    src = x[k * NCORES:(k + 1) * NCORES, s * CSP:(s + 1) * CSP].reshape(
        NCORES, CSP, 2, 128, 256)
    am = np.maximum(src.max(axis=-1), -src.min(axis=-1))  # row absmax, no temp
    sc = np.maximum(am * ISCALE, SMIN)                  # f32 scales
    np.multiply(src, (1.0 / sc)[..., None], out=tmp)
    tmp += 16.4995        # codes stay < 32 (consistent f32 scales), no clip
    cb[...] = tmp                                       # trunc -> round
    cv = cb.reshape(NCORES, CSP, 2, 128, 32, 8)
    pv = pkb.reshape(NCORES, CSP, 2, 128, 32, 5)
    c = [cv[..., i] for i in range(8)]
    pv[..., 0] = (c[0] << 3) | (c[1] >> 2)
    pv[..., 1] = (c[1] << 6) | (c[2] << 1) | (c[3] >> 4)
    pv[..., 2] = (c[3] << 4) | (c[4] >> 1)
    pv[..., 3] = (c[4] << 7) | (c[5] << 2) | (c[6] >> 3)
    pv[..., 4] = (c[6] << 5) | c[7]
    scb[...] = sc.transpose(0, 3, 1, 2).reshape(NCORES * 128, CSP, 2)
    return pkb, scb


def _unpack_quarter(x, ybuf, k, qtr, pk_np, s8_np):
    """Reconstruct y for chunk k, plane eighth `qtr` (8 channels)."""
    lut64, lut_fp8 = _state["lut64"], _state["lut_fp8"]
    cq = C // 8
    c64 = lut64[pk_np.reshape(-1)]
    codes = c64.view("<f4").reshape(NCORES, cq, 2, 128, 16, 16)
    scales = lut_fp8[s8_np.reshape(NCORES, C, 2, 128, 16).view(np.uint8)
                     [:, qtr * cq:(qtr + 1) * cq]]
    np.multiply(codes, scales[..., None], out=codes)
    o0 = qtr * cq
    xs = x[k * NCORES:(k + 1) * NCORES, o0:o0 + cq].reshape(
        NCORES, cq, 2, 128, 256)
    ysl = ybuf[k * NCORES:(k + 1) * NCORES, o0:o0 + cq].reshape(
        NCORES, cq, 2, 128, 256)
    np.add(xs, codes.reshape(NCORES, cq, 2, 128, 256), out=ysl)


def kernel(x, v, conv_w, tau):
    # one retry with a full rebuild: a transient relay/device error mid-call
    # would otherwise fail the whole run (NEFF compile is disk-cached, so a
    # rebuild costs seconds, not minutes)
    try:
        return _kernel(x, v, conv_w, tau)
    except Exception:
        _state.clear()
        return _kernel(x, v, conv_w, tau)


def _kernel(x, v, conv_w, tau):
    import jax
    from jax.sharding import PartitionSpec, NamedSharding

    x = np.asarray(x, dtype=np.float32)
    with _lock:
        _ensure_built()
        _ensure_consts(np.asarray(v, np.float32),
                       np.asarray(conv_w, np.float32),
                       np.asarray(tau, np.float32))
        fast, in_names, out_names, zeros_dev = _state["fast"]
        sh = NamedSharding(_state["mesh"], PartitionSpec("core"))
        consts_dev = _state["consts_dev"]

        i_pk = [out_names.index(f"pk{h}") for h in range(8)]
        i_s8 = out_names.index("s8")
        fetch_order = [i_s8] + i_pk

        # pack + async upload per chunk; dispatch exec as soon as its
        # inputs are queued so downloads start during chunk1 uploads
        dev_in = [[None] * NSPLIT for _ in range(NCHUNK)]
        dev_sc = [[None] * NSPLIT for _ in range(NCHUNK)]
        outs = []
        for k in range(NCHUNK):
            for s in range(NSPLIT):
                pkb, scb = _pack_chunk_split(x, k, s)
                dev_sc[k][s] = jax.device_put(scb, sh)
                dev_in[k][s] = jax.device_put(pkb, sh)
            args = []
            for n in in_names:
                if n.startswith("xp"):
                    args.append(dev_in[k][int(n[2:])])
                elif n.startswith("sc"):
                    args.append(dev_sc[k][int(n[2:])])
                else:
                    args.append(consts_dev[n])
            outs.append(fast(*args, *zeros_dev))
            for i in fetch_order:
                try:
                    outs[k][i].copy_to_host_async()
                except Exception:
                    pass

        ybuf = _state["ybuf"]
        for k in range(NCHUNK):
            s8_np = np.asarray(outs[k][i_s8])
            for qtr in range(8):
                pk_np = np.asarray(outs[k][i_pk[qtr]])
                _unpack_quarter(x, ybuf, k, qtr, pk_np, s8_np)
    return ybuf


# revision 6
# speedup vs baseline: 1.0053x; 1.0053x over previous
"""nn_HWTConv2D Trainium2 kernel, v2 — wire-optimized pipeline.

y = x + iHaar2d( sum_p SoftThresh( conv1x1_p( Haar2d(x) * v_p ), tau_p ) )

The axon tunnel (~45MB/s, shared half-duplex) dominates wall time, so v3:
  * uploads x as 5-bit codes packed 8/5bytes with per-row fp32 scales
    (42MB), split into 2 chunks x 4 channel-groups, each packed +
    device_put async so packing overlaps the wire;
  * keeps v/conv_w/tau + Haar matrices resident on device across calls
    (re-uploaded only if their bytes change);
  * downloads corr as int4 codes packed 2/byte with per-16-element fp8
    block scales (37.7MB total vs 67MB), quantized on device with
    magic-number round-to-nearest (bit-exact vs numpy, verified);
  * unpacks on host with a 256-entry float64-pair LUT (one gather) and
    fuses the residual add, overlapping the remaining downloads at
    8-channel granularity (16 download+unpack units per call).

Device program (per core, one batch per exec):
  S1/S2  forward Haar, data-stationary matmuls (fp8 data x bf16 HM^T).
  conv   channel mix via DMA-gathered [c|pix] tiles, one W^T per pod.
  thresh f5 = t - clip(t, +-tau) as min/mult ops.
  I_h/I_w inverse Haar; each [128,256] fp32 PSUM plane-half is then
         block-quantized: absmax/16 -> fp8 scale -> reciprocal ->
         q = RN(v/s)+8 in [0,15] -> pack hi*16+lo -> uint8 DMA.
"""

import threading

import numpy as np
import ml_dtypes

B, C, H, W, P = 16, 64, 256, 256, 2
NCORES = 8
NCHUNK = 2            # execs per call; chunk k = batches k*8 .. k*8+7
NSPLIT = 4            # xs channel-group splits per chunk
CSP = C // NSPLIT     # channels per split
BF16 = ml_dtypes.bfloat16
FP8 = ml_dtypes.float8_e4m3
NORM = float(1.0 / np.sqrt(2.0))
MAGIC = 8388608.0     # 2^23: (v + MAGIC) - MAGIC == round-to-nearest(v), v >= 0.25
MAGIC2 = 12582912.0   # 1.5*2^23: same trick valid for v >= -0.5 (unpack floors)
QSCALE = 1.0 / 7.49   # block absmax -> quant scale
SMIN = 2.0 ** -9      # fp8 subnormal floor for all-zero blocks
ISCALE = 1.0 / 15.49  # input row absmax -> 5-bit scale

_lock = threading.Lock()
_state: dict = {}


def _haar_matrix(n):
    # Orthonormal multilevel 1D Haar matrix: haar1d_fwd(x) == HM @ x.
    m = int(np.log2(n))
    hm = np.eye(n, dtype=np.float64)
    length = n
    for _ in range(m):
        lvl = np.eye(n, dtype=np.float64)
        half = length // 2
        blk = np.zeros((length, length), dtype=np.float64)
        for i in range(half):
            blk[i, 2 * i] = NORM
            blk[i, 2 * i + 1] = NORM
            blk[half + i, 2 * i] = NORM
            blk[half + i, 2 * i + 1] = -NORM
        lvl[:length, :length] = blk
        hm = lvl @ hm
        length //= 2
    return hm.astype(np.float32)


def _build_nc():
    import concourse.bacc as bacc
    import concourse.tile as tile
    from concourse import mybir
    from contextlib import ExitStack

    dt = mybir.dt
    alu = mybir.AluOpType
    nc = bacc.Bacc("TRN2", target_bir_lowering=False, debug=False)

    xps = [nc.dram_tensor(f"xp{s}", [CSP, 2, 128, 160], dt.uint8,
                          kind="ExternalInput") for s in range(NSPLIT)]
    scs = [nc.dram_tensor(f"sc{s}", [128, CSP, 2], dt.float32,
                          kind="ExternalInput") for s in range(NSPLIT)]
    hmt_s = nc.dram_tensor("hmt_s", [128, 2, 256], dt.bfloat16, kind="ExternalInput")
    hm_s = nc.dram_tensor("hm_s", [128, 2, 256], dt.bfloat16, kind="ExternalInput")
    wdup = nc.dram_tensor("wdup", [64, P, 64], dt.bfloat16, kind="ExternalInput")
    v_s = nc.dram_tensor("v_s", [128, P, 2, 256], dt.bfloat16, kind="ExternalInput")
    tau_s = nc.dram_tensor("tau_s", [128, P, 2, 256], dt.bfloat16, kind="ExternalInput")
    pks = [nc.dram_tensor(f"pk{h}", [C // 8, 2, 128, 128], dt.uint8,
                          kind="ExternalOutput") for h in range(8)]
    s8_out = nc.dram_tensor("s8", [C, 2, 128, 16], dt.float8e4, kind="ExternalOutput")

    with ExitStack() as ctx:
        tc = ctx.enter_context(tile.TileContext(nc))
        pc = ctx.enter_context(tc.tile_pool(name="consts", bufs=1))
        pw = ctx.enter_context(tc.tile_pool(name="work", bufs=1))
        pps = ctx.enter_context(tc.tile_pool(name="ps", bufs=4, space="PSUM"))
        pps3 = ctx.enter_context(tc.tile_pool(name="ps3", bufs=4, space="PSUM"))

        hmt_sb = pc.tile_from(hmt_s[:])
        hm_sb = pc.tile_from(hm_s[:])
        wdup_sb = pc.tile_from(wdup[:])
        v_sb = pc.tile_from(v_s[:])
        tau_sb = pc.tile_from(tau_s[:])

        f1 = pw.tile([128, 2, C, 256], dt.bfloat16, tag="f1")
        q = pw.tile([128, 2, C, 256], dt.bfloat16, tag="q")
        q2 = pw.tile([128, C, 256], dt.bfloat16, tag="q2")
        xcs = [pw.tile([128, 2, 256], dt.bfloat16, tag=f"xc{i}", name=f"xc{i}") for i in range(3)]
        # 6-bit input unpack tiles
        sc_sb = [pc.tile_from(scs[s][:], name=f"sc_sb{s}") for s in range(NSPLIT)]
        bus = [pw.tile([128, 2, 160], dt.uint8, tag=f"bu{i}", name=f"bu{i}") for i in range(2)]
        bfs = [pw.tile([128, 2, 160], dt.float32, tag=f"bfc{i}", name=f"bfc{i}") for i in range(2)]
        ufl = pw.tile([128, 32], dt.float32, tag="ufl")
        ut1 = pw.tile([128, 32], dt.float32, tag="ut1")
        ut3 = pw.tile([128, 32], dt.float32, tag="ut3")
        ut4 = pw.tile([128, 32], dt.float32, tag="ut4")
        ut6 = pw.tile([128, 32], dt.float32, tag="ut6")
        ur0 = pw.tile([128, 32], dt.float32, tag="ur0")
        ur1 = pw.tile([128, 32], dt.float32, tag="ur1")
        ur2 = pw.tile([128, 32], dt.float32, tag="ur2")
        ur3 = pw.tile([128, 32], dt.float32, tag="ur3")
        uc3h = pw.tile([128, 32], dt.float32, tag="uc3h")
        uc6h = pw.tile([128, 32], dt.float32, tag="uc6h")
        uc8s = [pw.tile([128, 32, 8], dt.float32, tag=f"uc8{i}", name=f"uc8{i}") for i in range(2)]
        r1s_ = [pw.tile([128, 256], dt.bfloat16, tag=f"r1{i}", name=f"r1{i}") for i in range(4)]
        ftcs = [pw.tile([64, 8 * 256], dt.bfloat16, tag=f"ftc{i}", name=f"ftc{i}") for i in range(2)]
        sgs = [pw.tile([64, 8 * 256], dt.bfloat16, tag=f"sg{i}", name=f"sg{i}") for i in range(2)]
        tts = [pw.tile([128, 8, 256], dt.bfloat16, tag="tt0", name="tt0")]
        mms = [pw.tile([128, 8, 256], dt.bfloat16, tag="mm0", name="mm0")]
        t2s = [pw.tile([128, 2, 256], dt.bfloat16, tag=f"t2{i}", name=f"t2{i}") for i in range(2)]
        # quantization work tiles (rotated)
        ams = [pw.tile([128, 16], dt.float32, tag=f"am{i}", name=f"am{i}") for i in range(2)]
        s8s = [pw.tile([128, 16], dt.float8e4, tag=f"s8{i}", name=f"s8{i}") for i in range(2)]
        sbs = [pw.tile([128, 16], dt.float32, tag=f"sb{i}", name=f"sb{i}") for i in range(2)]
        qts = [pw.tile([128, 16, 16], dt.float32, tag=f"qt{i}", name=f"qt{i}") for i in range(2)]
        pkts = [pw.tile([128, 128], dt.uint8, tag=f"pkt{i}", name=f"pkt{i}") for i in range(2)]

        # ---- forward transform: per-plane, 6-bit unpack + fused S1+S2 ----
        for c in range(C):
            xc = xcs[c % 3]
            bu = bus[c % 2]
            bf = bfs[c % 2]
            c8 = uc8s[c % 2]
            xsrc = xps[c // CSP]
            for hh in range(2):
                nc.gpsimd.dma_start(bu[:, hh, :], xsrc[c % CSP, hh])
            nc.any.tensor_copy(bf[:], bu[:])          # u8 -> fp32 exact
            stt = nc.vector.scalar_tensor_tensor

            def fl(dst, srcap, inv, d):
                # dst = floor(srcap * inv), via exact-offset + magic RN
                nc.vector.tensor_scalar(ufl[:], srcap, inv, -d, alu.mult, alu.add)
                nc.vector.tensor_scalar(dst, ufl[:], MAGIC2, -MAGIC2, alu.add, alu.add)

            for hh in range(2):
                bv = bf[:, hh, :].rearrange("p (g five) -> p g five", five=5)
                B0, B1, B2, B3, B4 = (bv[:, :, i] for i in range(5))
                fl(c8[:, :, 0], B0, 0.125, 0.4375)                        # c0
                stt(ur0[:], c8[:, :, 0], -8.0, B0, alu.mult, alu.add)
                fl(ut1[:], B1, 1.0 / 64.0, 0.4921875)                     # c1&3
                stt(c8[:, :, 1], ur0[:], 4.0, ut1[:], alu.mult, alu.add)  # c1
                stt(ur1[:], ut1[:], -64.0, B1, alu.mult, alu.add)
                fl(c8[:, :, 2], ur1[:], 0.5, 0.25)                        # c2
                stt(uc3h[:], c8[:, :, 2], -2.0, ur1[:], alu.mult, alu.add)
                fl(ut3[:], B2, 1.0 / 16.0, 0.46875)                       # c3&15
                stt(c8[:, :, 3], uc3h[:], 16.0, ut3[:], alu.mult, alu.add)
                stt(ur2[:], ut3[:], -16.0, B2, alu.mult, alu.add)
                fl(ut4[:], B3, 1.0 / 128.0, 0.498046875)                  # c4&1
                stt(c8[:, :, 4], ur2[:], 2.0, ut4[:], alu.mult, alu.add)
                stt(ur3[:], ut4[:], -128.0, B3, alu.mult, alu.add)
                fl(c8[:, :, 5], ur3[:], 0.25, 0.375)                      # c5
                stt(uc6h[:], c8[:, :, 5], -4.0, ur3[:], alu.mult, alu.add)
                fl(ut6[:], B4, 1.0 / 32.0, 0.484375)                      # c6&7
                stt(c8[:, :, 6], uc6h[:], 8.0, ut6[:], alu.mult, alu.add)
                stt(c8[:, :, 7], ut6[:], -32.0, B4, alu.mult, alu.add)    # c7
                # dequant: xc = (codes - 16) * scale_row
                cvf = c8[:].rearrange("p g e -> p (g e)")
                scb = sc_sb[c // CSP][:, c % CSP, hh].unsqueeze(1).broadcast_to([128, 256])
                stt(xc[:, hh, :], cvf, -16.0, scb, alu.add, alu.mult)
            r1s = []
            for wh in range(2):
                ps1 = pps.tile([128, 256], dt.float32, tag="ps")
                nc.tensor.matmul(ps1[:], xc[:, 0, wh * 128:(wh + 1) * 128],
                                 hmt_sb[:, 0, :], start=True, stop=False)
                nc.tensor.matmul(ps1[:], xc[:, 1, wh * 128:(wh + 1) * 128],
                                 hmt_sb[:, 1, :], start=False, stop=True)
                r1 = r1s_[(2 * c + wh) % 4]
                nc.any.tensor_copy(r1[:], ps1[:])
                r1s.append(r1)
            for hph in range(2):
                ps2 = pps.tile([128, 256], dt.float32, tag="ps")
                nc.tensor.matmul(ps2[:], r1s[0][:, hph * 128:(hph + 1) * 128],
                                 hmt_sb[:, 0, :], start=True, stop=False)
                nc.tensor.matmul(ps2[:], r1s[1][:, hph * 128:(hph + 1) * 128],
                                 hmt_sb[:, 1, :], start=False, stop=True)
                nc.any.tensor_copy(f1[:, hph, c, :], ps2[:])

        # ---- conv (channel mix) + soft-threshold ----
        for hph in range(2):
            for chk in range(16):
                ftc = ftcs[chk % 2]
                ftv = ftc[:].rearrange("c (hl w) -> c hl w", hl=8)
                for hl in range(8):
                    row = chk * 8 + hl
                    nc.gpsimd.dma_start(ftv[:, hl, :], f1[row:row + 1, hph, :, :])
                for pod in range(P):
                    sg = sgs[pod]
                    for q4 in range(4):
                        ps3 = pps3.tile([64, 512], dt.float32, tag="ps3")
                        nc.tensor.matmul(ps3[:], wdup_sb[:, pod, :],
                                         ftc[:, q4 * 512:(q4 + 1) * 512],
                                         start=True, stop=True)
                        nc.any.tensor_copy(sg[:, q4 * 512:(q4 + 1) * 512], ps3[:])
                    dst = q if pod == 0 else q2
                    sgv = sg[:].rearrange("o (hl w) -> o hl w", hl=8)
                    for hl in range(8):
                        row = chk * 8 + hl
                        drow = (dst[row:row + 1, hph, :, :] if pod == 0
                                else dst[row:row + 1, :, :])
                        nc.gpsimd.dma_start(drow, sgv[:, hl, :])
            for ch2 in range(8):
                osl = slice(ch2 * 8, (ch2 + 1) * 8)
                qs = q[:, hph, osl, :]
                q2s = q2[:, osl, :]
                t = tts[0]
                m = mms[0]
                vb0 = v_sb[:, 0, hph, :].unsqueeze(1).broadcast_to([128, 8, 256])
                tb0 = tau_sb[:, 0, hph, :].unsqueeze(1).broadcast_to([128, 8, 256])
                vb1 = v_sb[:, 1, hph, :].unsqueeze(1).broadcast_to([128, 8, 256])
                tb1 = tau_sb[:, 1, hph, :].unsqueeze(1).broadcast_to([128, 8, 256])
                # pod0, in place: q <- t + min(-min(t,tau), tau) = t - clip(t)
                nc.vector.tensor_tensor(t[:], qs, vb0, alu.mult)
                nc.vector.tensor_tensor(m[:], t[:], tb0, alu.min)
                nc.vector.scalar_tensor_tensor(m[:], m[:], -1.0, tb0, alu.mult, alu.min)
                nc.vector.tensor_tensor(qs, t[:], m[:], alu.add)
                # pod1, accumulate into q
                nc.vector.tensor_tensor(t[:], q2s, vb1, alu.mult)
                nc.vector.tensor_tensor(m[:], t[:], tb1, alu.min)
                nc.vector.scalar_tensor_tensor(m[:], m[:], -1.0, tb1, alu.mult, alu.min)
                nc.vector.tensor_tensor(t[:], t[:], m[:], alu.add)
                nc.vector.tensor_tensor(qs, qs, t[:], alu.add)

        # ---- inverse transform + int4-blk16 quantization, per plane ----
        for o in range(C):
            t2 = t2s[o % 2]
            for wph in range(2):
                ps = pps.tile([128, 256], dt.float32, tag="ps")
                nc.tensor.matmul(ps[:], q[:, 0, o, wph * 128:(wph + 1) * 128],
                                 hm_sb[:, 0, :], start=True, stop=False)
                nc.tensor.matmul(ps[:], q[:, 1, o, wph * 128:(wph + 1) * 128],
                                 hm_sb[:, 1, :], start=False, stop=True)
                nc.any.tensor_copy(t2[:, wph, :], ps[:])
            for hh in range(2):
                ps = pps.tile([128, 256], dt.float32, tag="ps")
                nc.tensor.matmul(ps[:], t2[:, 0, hh * 128:(hh + 1) * 128],
                                 hm_sb[:, 0, :], start=True, stop=False)
                nc.tensor.matmul(ps[:], t2[:, 1, hh * 128:(hh + 1) * 128],
                                 hm_sb[:, 1, :], start=False, stop=True)
                k = (2 * o + hh) % 2
                am, s8, sb, qt, pkt = ams[k], s8s[k], sbs[k], qts[k], pkts[k]
                cv = ps[:].rearrange("p (b e) -> p b e", b=16)
                nc.vector.tensor_reduce(am[:], cv, axis=mybir.AxisListType.X,
                                        op=alu.max, apply_absolute_value=True)
                nc.vector.tensor_scalar_mul(am[:], am[:], QSCALE)
                nc.vector.tensor_scalar_max(am[:], am[:], SMIN)
                nc.any.tensor_copy(s8[:], am[:])     # fp32 -> fp8 (wire scale)
                nc.any.tensor_copy(sb[:], s8[:])     # fp8 -> fp32 (consistent)
                nc.vector.reciprocal(sb[:], sb[:])
                sb_b = sb[:].unsqueeze(2).broadcast_to([128, 16, 16])
                nc.vector.tensor_tensor(qt[:], cv, sb_b, alu.mult)
                nc.vector.tensor_scalar_add(qt[:], qt[:], 8.0 + MAGIC)
                nc.vector.tensor_scalar_add(qt[:], qt[:], -MAGIC)
                nc.vector.tensor_scalar_min(qt[:], qt[:], 15.0)
                qv = qt[:].rearrange("p b e -> p (b e)").rearrange(
                    "p (k two) -> p k two", two=2)
                nc.vector.scalar_tensor_tensor(pkt[:], qv[:, :, 0], 16.0,
                                               qv[:, :, 1], alu.mult, alu.add)
                nc.gpsimd.dma_start(pks[o // (C // 8)][o % (C // 8), hh], pkt[:])
                nc.gpsimd.dma_start(s8_out[o, hh], s8[:])

    nc.compile()
    nc.finalize()
    return nc


def _prep_consts(v, conv_w, tau):
    hm = _haar_matrix(H)
    hmt = np.ascontiguousarray(hm.T)
    hmt_s = np.ascontiguousarray(hmt.reshape(2, 128, 256).transpose(1, 0, 2)).astype(BF16)
    hm_s = np.ascontiguousarray(hm.reshape(2, 128, 256).transpose(1, 0, 2)).astype(BF16)
    wdup = np.ascontiguousarray(conv_w.transpose(2, 0, 1)).astype(BF16)  # [c, pod, o]
    v_s = np.ascontiguousarray(
        v.reshape(P, 2, 128, 256).transpose(2, 0, 1, 3)).astype(BF16)
    tau_s = np.ascontiguousarray(
        tau.reshape(P, 2, 128, 256).transpose(2, 0, 1, 3)).astype(BF16)
    return {"hmt_s": hmt_s, "hm_s": hm_s, "wdup": wdup, "v_s": v_s, "tau_s": tau_s}


def _build_fast(nc, mesh):
    """jit(shard_map) executing the bass NEFF on 8 cores (bass2jax custom
    call), with device-resident zero output buffers."""
    import jax
    from jax.sharding import PartitionSpec, NamedSharding
    try:
        from jax.experimental.shard_map import shard_map
    except ImportError:
        from jax.shard_map import shard_map
    from concourse import bass2jax, mybir

    bass2jax.install_neuronx_cc_hook()

    pname = nc.partition_id_tensor.name if nc.partition_id_tensor else None
    in_names, out_names, out_avals = [], [], []
    for alloc in nc.m.functions[0].allocations:
        if not isinstance(alloc, mybir.MemoryLocationSet):
            continue
        name = alloc.memorylocations[0].name
        if alloc.kind == "ExternalInput":
            if name != pname:
                in_names.append(name)
        elif alloc.kind == "ExternalOutput":
            out_names.append(name)
            out_avals.append(jax.core.ShapedArray(
                tuple(alloc.tensor_shape), mybir.dt.np(alloc.dtype)))

    bind_names = list(in_names) + list(out_names)
    if pname is not None:
        bind_names.append(pname)

    def _body(*args):
        operands = list(args)
        if pname is not None:
            operands.append(bass2jax.partition_id_tensor())
        outs = bass2jax._bass_exec_p.bind(
            *operands,
            out_avals=tuple(out_avals),
            in_names=tuple(bind_names),
            out_names=tuple(out_names),
            lowering_input_output_aliases=(),
            sim_require_finite=True,
            sim_require_nnan=True,
            nc=nc,
        )
        return tuple(outs)

    n_args = len(in_names) + len(out_avals)
    fast = jax.jit(shard_map(
        _body, mesh=mesh,
        in_specs=(PartitionSpec("core"),) * n_args,
        out_specs=(PartitionSpec("core"),) * len(out_names),
        check_rep=False))
    sh = NamedSharding(mesh, PartitionSpec("core"))
    zeros_dev = [
        jax.device_put(
            np.zeros((NCORES * a.shape[0], *a.shape[1:]), a.dtype), sh)
        for a in out_avals
    ]
    for z in zeros_dev:
        z.block_until_ready()
    return fast, in_names, out_names, zeros_dev


def _make_luts():
    # packed byte -> (hi_code-8, lo_code-8) as adjacent fp32 in one f64 slot
    idx = np.arange(256)
    lut64 = np.empty(256, dtype="<f8")
    lv = lut64.view("<f4").reshape(256, 2)
    lv[:, 0] = (idx >> 4) - 8.0
    lv[:, 1] = (idx & 15) - 8.0
    # fp8 byte -> fp32 scale
    lut_fp8 = np.arange(256, dtype=np.uint8).view(FP8).astype(np.float32)
    return lut64, lut_fp8


def _ensure_built():
    import jax
    from jax.sharding import Mesh
    if "fast" in _state:
        return
    nc = _build_nc()
    mesh = Mesh(np.asarray(jax.devices()[:NCORES]), ("core",))
    _state["mesh"] = mesh
    _state["fast"] = _build_fast(nc, mesh)
    _state["lut64"], _state["lut_fp8"] = _make_luts()
    _state["xh"] = [[np.empty((NCORES * CSP, 2, 128, 160), dtype=np.uint8)
                     for _ in range(NSPLIT)] for _ in range(NCHUNK)]
    _state["sch"] = [[np.empty((NCORES * 128, CSP, 2), dtype=np.float32)
                      for _ in range(NSPLIT)] for _ in range(NCHUNK)]
    _state["qtmp"] = np.empty((NCORES, CSP, 2, 128, 256), dtype=np.float32)
    _state["ctmp"] = np.empty((NCORES, CSP, 2, 128, 256), dtype=np.uint8)
    _state["ybuf"] = np.empty((B, C, H, W), dtype=np.float32)


def _ensure_consts(v, conv_w, tau):
    import jax
    from jax.sharding import PartitionSpec, NamedSharding
    key = (v.tobytes(), conv_w.tobytes(), tau.tobytes())
    cached = _state.get("consts_key")
    if cached is not None and cached == key:
        return
    consts = _prep_consts(v, conv_w, tau)
    sh = NamedSharding(_state["mesh"], PartitionSpec("core"))
    dev = {}
    for n, a in consts.items():
        g = np.broadcast_to(a[None], (NCORES,) + a.shape).reshape(
            (NCORES * a.shape[0],) + a.shape[1:])
        dev[n] = jax.device_put(np.ascontiguousarray(g), sh)
    for d in dev.values():
        d.block_until_ready()
    _state["consts_dev"] = dev
    _state["consts_key"] = key


def _pack_chunk_split(x, k, s):
    """6-bit rowscale pack of batches k*8.., channels s*CSP.. ->
    (packed [8*CSP,2,128,192] u8, scales [8*128,CSP,2] f32)."""
    pkb = _state["xh"][k][s]
    scb = _state["sch"][k][s]
    tmp = _state["qtmp"]
    cb = _state["ctmp"]
    src = x[k * NCORES:(k + 1) * NCORES, s * CSP:(s + 1) * CSP].reshape(
        NCORES, CSP, 2, 128, 256)
    am = np.maximum(src.max(axis=-1), -src.min(axis=-1))  # row absmax, no temp
    sc = np.maximum(am * ISCALE, SMIN)                  # f32 scales
    np.multiply(src, (1.0 / sc)[..., None], out=tmp)
    tmp += 16.4995        # codes stay < 32 (consistent f32 scales), no clip
    cb[...] = tmp                                       # trunc -> round
    cv = cb.reshape(NCORES, CSP, 2, 128, 32, 8)
    pv = pkb.reshape(NCORES, CSP, 2, 128, 32, 5)
    c = [cv[..., i] for i in range(8)]
    pv[..., 0] = (c[0] << 3) | (c[1] >> 2)
    pv[..., 1] = (c[1] << 6) | (c[2] << 1) | (c[3] >> 4)
    pv[..., 2] = (c[3] << 4) | (c[4] >> 1)
    pv[..., 3] = (c[4] << 7) | (c[5] << 2) | (c[6] >> 3)
    pv[..., 4] = (c[6] << 5) | c[7]
    scb[...] = sc.transpose(0, 3, 1, 2).reshape(NCORES * 128, CSP, 2)
    return pkb, scb


def _unpack_quarter(x, ybuf, k, qtr, pk_np, s8_np):
    """Reconstruct y for chunk k, plane eighth `qtr` (8 channels)."""
    lut64, lut_fp8 = _state["lut64"], _state["lut_fp8"]
    cq = C // 8
    c64 = lut64[pk_np.reshape(-1)]
    codes = c64.view("<f4").reshape(NCORES, cq, 2, 128, 16, 16)
    scales = lut_fp8[s8_np.reshape(NCORES, C, 2, 128, 16).view(np.uint8)
                     [:, qtr * cq:(qtr + 1) * cq]]
    np.multiply(codes, scales[..., None], out=codes)
    o0 = qtr * cq
    xs = x[k * NCORES:(k + 1) * NCORES, o0:o0 + cq].reshape(
        NCORES, cq, 2, 128, 256)
    ysl = ybuf[k * NCORES:(k + 1) * NCORES, o0:o0 + cq].reshape(
        NCORES, cq, 2, 128, 256)
    np.add(xs, codes.reshape(NCORES, cq, 2, 128, 256), out=ysl)


def kernel(x, v, conv_w, tau):
    # retries with a full rebuild: a transient relay/device error mid-call
    # would otherwise fail the whole run (NEFF compile is disk-cached, so a
    # rebuild costs seconds, not minutes)
    for attempt in range(3):
        try:
            return _kernel(x, v, conv_w, tau)
        except Exception:
            _state.clear()
            if attempt == 2:
                raise
    return None  # unreachable


def _kernel(x, v, conv_w, tau):
    import jax
    from jax.sharding import PartitionSpec, NamedSharding

    x = np.asarray(x, dtype=np.float32)
    with _lock:
        _ensure_built()
        _ensure_consts(np.asarray(v, np.float32),
                       np.asarray(conv_w, np.float32),
                       np.asarray(tau, np.float32))
        fast, in_names, out_names, zeros_dev = _state["fast"]
        sh = NamedSharding(_state["mesh"], PartitionSpec("core"))
        consts_dev = _state["consts_dev"]

        i_pk = [out_names.index(f"pk{h}") for h in range(8)]
        i_s8 = out_names.index("s8")
        fetch_order = [i_s8] + i_pk

        # pack + async upload per chunk; dispatch exec as soon as its
        # inputs are queued so downloads start during chunk1 uploads
        dev_in = [[None] * NSPLIT for _ in range(NCHUNK)]
        dev_sc = [[None] * NSPLIT for _ in range(NCHUNK)]
        outs = []
        for k in range(NCHUNK):
            for s in range(NSPLIT):
                pkb, scb = _pack_chunk_split(x, k, s)
                dev_sc[k][s] = jax.device_put(scb, sh)
                dev_in[k][s] = jax.device_put(pkb, sh)
            args = []
            for n in in_names:
                if n.startswith("xp"):
                    args.append(dev_in[k][int(n[2:])])
                elif n.startswith("sc"):
                    args.append(dev_sc[k][int(n[2:])])
                else:
                    args.append(consts_dev[n])
            outs.append(fast(*args, *zeros_dev))
            for i in fetch_order:
                try:
                    outs[k][i].copy_to_host_async()
                except Exception:
                    pass

        ybuf = _state["ybuf"]
        for k in range(NCHUNK):
            s8_np = np.asarray(outs[k][i_s8])
            for qtr in range(8):
                pk_np = np.asarray(outs[k][i_pk[qtr]])
                _unpack_quarter(x, ybuf, k, qtr, pk_np, s8_np)
    return ybuf


# revision 7
# speedup vs baseline: 1.0283x; 1.0229x over previous
"""nn_HWTConv2D Trainium2 kernel, v2 — wire-optimized pipeline.

y = x + iHaar2d( sum_p SoftThresh( conv1x1_p( Haar2d(x) * v_p ), tau_p ) )

The axon tunnel (~45MB/s, shared half-duplex) dominates wall time, so v3:
  * uploads x as 5-bit codes packed 8/5bytes with per-row fp32 scales
    (42.5MB incl. fp8 row scales), split into 2 chunks x 4 channel-groups, each packed +
    device_put async so packing overlaps the wire;
  * keeps v/conv_w/tau + Haar matrices resident on device across calls
    (re-uploaded only if their bytes change);
  * downloads corr as int4 codes packed 2/byte with per-16-element fp8
    block scales (37.7MB total vs 67MB), quantized on device with
    magic-number round-to-nearest (bit-exact vs numpy, verified);
  * unpacks on host with a 256-entry float64-pair LUT (one gather) and
    fuses the residual add, overlapping the remaining downloads.

Device program (per core, one batch per exec):
  S1/S2  forward Haar, data-stationary matmuls (fp8 data x bf16 HM^T).
  conv   channel mix via DMA-gathered [c|pix] tiles, one W^T per pod.
  thresh f5 = t - clip(t, +-tau) as min/mult ops.
  I_h/I_w inverse Haar; each [128,256] fp32 PSUM plane-half is then
         block-quantized: absmax/16 -> fp8 scale -> reciprocal ->
         q = RN(v/s)+8 in [0,15] -> pack hi*16+lo -> uint8 DMA.
"""

import threading

import numpy as np
import ml_dtypes

B, C, H, W, P = 16, 64, 256, 256, 2
NCORES = 8
NCHUNK = 2            # execs per call; chunk k = batches k*8 .. k*8+7
NSPLIT = 4            # xs channel-group splits per chunk
CSP = C // NSPLIT     # channels per split
BF16 = ml_dtypes.bfloat16
FP8 = ml_dtypes.float8_e4m3
NORM = float(1.0 / np.sqrt(2.0))
MAGIC = 8388608.0     # 2^23: (v + MAGIC) - MAGIC == round-to-nearest(v), v >= 0.25
MAGIC2 = 12582912.0   # 1.5*2^23: same trick valid for v >= -0.5 (unpack floors)
QSCALE = 1.0 / 7.49   # block absmax -> quant scale
SMIN = 2.0 ** -9      # fp8 subnormal floor for all-zero blocks
ISCALE = 1.0 / 15.49  # input row absmax -> 5-bit scale

_lock = threading.Lock()
_state: dict = {}


def _haar_matrix(n):
    # Orthonormal multilevel 1D Haar matrix: haar1d_fwd(x) == HM @ x.
    m = int(np.log2(n))
    hm = np.eye(n, dtype=np.float64)
    length = n
    for _ in range(m):
        lvl = np.eye(n, dtype=np.float64)
        half = length // 2
        blk = np.zeros((length, length), dtype=np.float64)
        for i in range(half):
            blk[i, 2 * i] = NORM
            blk[i, 2 * i + 1] = NORM
            blk[half + i, 2 * i] = NORM
            blk[half + i, 2 * i + 1] = -NORM
        lvl[:length, :length] = blk
        hm = lvl @ hm
        length //= 2
    return hm.astype(np.float32)


def _build_nc():
    import concourse.bacc as bacc
    import concourse.tile as tile
    from concourse import mybir
    from contextlib import ExitStack

    dt = mybir.dt
    alu = mybir.AluOpType
    nc = bacc.Bacc("TRN2", target_bir_lowering=False, debug=False)

    xps = [nc.dram_tensor(f"xp{s}", [CSP, 2, 128, 160], dt.uint8,
                          kind="ExternalInput") for s in range(NSPLIT)]
    scs = [nc.dram_tensor(f"sc{s}", [128, CSP, 2], dt.float8e4,
                          kind="ExternalInput") for s in range(NSPLIT)]
    hmt_s = nc.dram_tensor("hmt_s", [128, 2, 256], dt.bfloat16, kind="ExternalInput")
    hm_s = nc.dram_tensor("hm_s", [128, 2, 256], dt.bfloat16, kind="ExternalInput")
    wdup = nc.dram_tensor("wdup", [64, P, 64], dt.bfloat16, kind="ExternalInput")
    v_s = nc.dram_tensor("v_s", [128, P, 2, 256], dt.bfloat16, kind="ExternalInput")
    tau_s = nc.dram_tensor("tau_s", [128, P, 2, 256], dt.bfloat16, kind="ExternalInput")
    pks = [nc.dram_tensor(f"pk{h}", [C // 8, 2, 128, 128], dt.uint8,
                          kind="ExternalOutput") for h in range(8)]
    s8_out = nc.dram_tensor("s8", [C, 2, 128, 16], dt.float8e4, kind="ExternalOutput")

    with ExitStack() as ctx:
        tc = ctx.enter_context(tile.TileContext(nc))
        pc = ctx.enter_context(tc.tile_pool(name="consts", bufs=1))
        pw = ctx.enter_context(tc.tile_pool(name="work", bufs=1))
        pps = ctx.enter_context(tc.tile_pool(name="ps", bufs=4, space="PSUM"))
        pps3 = ctx.enter_context(tc.tile_pool(name="ps3", bufs=4, space="PSUM"))

        hmt_sb = pc.tile_from(hmt_s[:])
        hm_sb = pc.tile_from(hm_s[:])
        wdup_sb = pc.tile_from(wdup[:])
        v_sb = pc.tile_from(v_s[:])
        tau_sb = pc.tile_from(tau_s[:])

        f1 = pw.tile([128, 2, C, 256], dt.bfloat16, tag="f1")
        q = pw.tile([128, 2, C, 256], dt.bfloat16, tag="q")
        q2 = pw.tile([128, C, 256], dt.bfloat16, tag="q2")
        xcs = [pw.tile([128, 2, 256], dt.bfloat16, tag=f"xc{i}", name=f"xc{i}") for i in range(3)]
        # 6-bit input unpack tiles
        sc_s8 = [pc.tile_from(scs[s][:], name=f"sc_s8{s}") for s in range(NSPLIT)]
        sc_sb = [pw.tile([128, CSP, 2], dt.float32, tag=f"scf{s}", name=f"scf{s}")
                 for s in range(NSPLIT)]
        for s in range(NSPLIT):
            nc.any.tensor_copy(sc_sb[s][:], sc_s8[s][:])   # fp8 -> f32 scales
        bus = [pw.tile([128, 2, 160], dt.uint8, tag=f"bu{i}", name=f"bu{i}") for i in range(2)]
        bfs = [pw.tile([128, 2, 160], dt.float32, tag=f"bfc{i}", name=f"bfc{i}") for i in range(2)]
        ufl = pw.tile([128, 32], dt.float32, tag="ufl")
        ut1 = pw.tile([128, 32], dt.float32, tag="ut1")
        ut3 = pw.tile([128, 32], dt.float32, tag="ut3")
        ut4 = pw.tile([128, 32], dt.float32, tag="ut4")
        ut6 = pw.tile([128, 32], dt.float32, tag="ut6")
        ur0 = pw.tile([128, 32], dt.float32, tag="ur0")
        ur1 = pw.tile([128, 32], dt.float32, tag="ur1")
        ur2 = pw.tile([128, 32], dt.float32, tag="ur2")
        ur3 = pw.tile([128, 32], dt.float32, tag="ur3")
        uc3h = pw.tile([128, 32], dt.float32, tag="uc3h")
        uc6h = pw.tile([128, 32], dt.float32, tag="uc6h")
        uc8s = [pw.tile([128, 32, 8], dt.float32, tag=f"uc8{i}", name=f"uc8{i}") for i in range(2)]
        r1s_ = [pw.tile([128, 256], dt.bfloat16, tag=f"r1{i}", name=f"r1{i}") for i in range(4)]
        ftcs = [pw.tile([64, 8 * 256], dt.bfloat16, tag=f"ftc{i}", name=f"ftc{i}") for i in range(2)]
        sgs = [pw.tile([64, 8 * 256], dt.bfloat16, tag=f"sg{i}", name=f"sg{i}") for i in range(2)]
        tts = [pw.tile([128, 8, 256], dt.bfloat16, tag="tt0", name="tt0")]
        mms = [pw.tile([128, 8, 256], dt.bfloat16, tag="mm0", name="mm0")]
        t2s = [pw.tile([128, 2, 256], dt.bfloat16, tag=f"t2{i}", name=f"t2{i}") for i in range(2)]
        # quantization work tiles (rotated)
        ams = [pw.tile([128, 16], dt.float32, tag=f"am{i}", name=f"am{i}") for i in range(2)]
        s8s = [pw.tile([128, 16], dt.float8e4, tag=f"s8{i}", name=f"s8{i}") for i in range(2)]
        sbs = [pw.tile([128, 16], dt.float32, tag=f"sb{i}", name=f"sb{i}") for i in range(2)]
        qts = [pw.tile([128, 16, 16], dt.float32, tag=f"qt{i}", name=f"qt{i}") for i in range(2)]
        pkts = [pw.tile([128, 128], dt.uint8, tag=f"pkt{i}", name=f"pkt{i}") for i in range(2)]

        # ---- forward transform: per-plane, 6-bit unpack + fused S1+S2 ----
        for c in range(C):
            xc = xcs[c % 3]
            bu = bus[c % 2]
            bf = bfs[c % 2]
            c8 = uc8s[c % 2]
            xsrc = xps[c // CSP]
            for hh in range(2):
                nc.gpsimd.dma_start(bu[:, hh, :], xsrc[c % CSP, hh])
            nc.any.tensor_copy(bf[:], bu[:])          # u8 -> fp32 exact
            stt = nc.vector.scalar_tensor_tensor

            def fl(dst, srcap, inv, d):
                # dst = floor(srcap * inv), via exact-offset + magic RN
                nc.vector.tensor_scalar(ufl[:], srcap, inv, -d, alu.mult, alu.add)
                nc.vector.tensor_scalar(dst, ufl[:], MAGIC2, -MAGIC2, alu.add, alu.add)

            for hh in range(2):
                bv = bf[:, hh, :].rearrange("p (g five) -> p g five", five=5)
                B0, B1, B2, B3, B4 = (bv[:, :, i] for i in range(5))
                fl(c8[:, :, 0], B0, 0.125, 0.4375)                        # c0
                stt(ur0[:], c8[:, :, 0], -8.0, B0, alu.mult, alu.add)
                fl(ut1[:], B1, 1.0 / 64.0, 0.4921875)                     # c1&3
                stt(c8[:, :, 1], ur0[:], 4.0, ut1[:], alu.mult, alu.add)  # c1
                stt(ur1[:], ut1[:], -64.0, B1, alu.mult, alu.add)
                fl(c8[:, :, 2], ur1[:], 0.5, 0.25)                        # c2
                stt(uc3h[:], c8[:, :, 2], -2.0, ur1[:], alu.mult, alu.add)
                fl(ut3[:], B2, 1.0 / 16.0, 0.46875)                       # c3&15
                stt(c8[:, :, 3], uc3h[:], 16.0, ut3[:], alu.mult, alu.add)
                stt(ur2[:], ut3[:], -16.0, B2, alu.mult, alu.add)
                fl(ut4[:], B3, 1.0 / 128.0, 0.498046875)                  # c4&1
                stt(c8[:, :, 4], ur2[:], 2.0, ut4[:], alu.mult, alu.add)
                stt(ur3[:], ut4[:], -128.0, B3, alu.mult, alu.add)
                fl(c8[:, :, 5], ur3[:], 0.25, 0.375)                      # c5
                stt(uc6h[:], c8[:, :, 5], -4.0, ur3[:], alu.mult, alu.add)
                fl(ut6[:], B4, 1.0 / 32.0, 0.484375)                      # c6&7
                stt(c8[:, :, 6], uc6h[:], 8.0, ut6[:], alu.mult, alu.add)
                stt(c8[:, :, 7], ut6[:], -32.0, B4, alu.mult, alu.add)    # c7
                # dequant: xc = (codes - 16) * scale_row
                cvf = c8[:].rearrange("p g e -> p (g e)")
                scb = sc_sb[c // CSP][:, c % CSP, hh].unsqueeze(1).broadcast_to([128, 256])
                stt(xc[:, hh, :], cvf, -16.0, scb, alu.add, alu.mult)
            r1s = []
            for wh in range(2):
                ps1 = pps.tile([128, 256], dt.float32, tag="ps")
                nc.tensor.matmul(ps1[:], xc[:, 0, wh * 128:(wh + 1) * 128],
                                 hmt_sb[:, 0, :], start=True, stop=False)
                nc.tensor.matmul(ps1[:], xc[:, 1, wh * 128:(wh + 1) * 128],
                                 hmt_sb[:, 1, :], start=False, stop=True)
                r1 = r1s_[(2 * c + wh) % 4]
                nc.any.tensor_copy(r1[:], ps1[:])
                r1s.append(r1)
            for hph in range(2):
                ps2 = pps.tile([128, 256], dt.float32, tag="ps")
                nc.tensor.matmul(ps2[:], r1s[0][:, hph * 128:(hph + 1) * 128],
                                 hmt_sb[:, 0, :], start=True, stop=False)
                nc.tensor.matmul(ps2[:], r1s[1][:, hph * 128:(hph + 1) * 128],
                                 hmt_sb[:, 1, :], start=False, stop=True)
                nc.any.tensor_copy(f1[:, hph, c, :], ps2[:])

        # ---- conv (channel mix) + soft-threshold ----
        for hph in range(2):
            for chk in range(16):
                ftc = ftcs[chk % 2]
                ftv = ftc[:].rearrange("c (hl w) -> c hl w", hl=8)
                for hl in range(8):
                    row = chk * 8 + hl
                    nc.gpsimd.dma_start(ftv[:, hl, :], f1[row:row + 1, hph, :, :])
                for pod in range(P):
                    sg = sgs[pod]
                    for q4 in range(4):
                        ps3 = pps3.tile([64, 512], dt.float32, tag="ps3")
                        nc.tensor.matmul(ps3[:], wdup_sb[:, pod, :],
                                         ftc[:, q4 * 512:(q4 + 1) * 512],
                                         start=True, stop=True)
                        nc.any.tensor_copy(sg[:, q4 * 512:(q4 + 1) * 512], ps3[:])
                    dst = q if pod == 0 else q2
                    sgv = sg[:].rearrange("o (hl w) -> o hl w", hl=8)
                    for hl in range(8):
                        row = chk * 8 + hl
                        drow = (dst[row:row + 1, hph, :, :] if pod == 0
                                else dst[row:row + 1, :, :])
                        nc.gpsimd.dma_start(drow, sgv[:, hl, :])
            for ch2 in range(8):
                osl = slice(ch2 * 8, (ch2 + 1) * 8)
                qs = q[:, hph, osl, :]
                q2s = q2[:, osl, :]
                t = tts[0]
                m = mms[0]
                vb0 = v_sb[:, 0, hph, :].unsqueeze(1).broadcast_to([128, 8, 256])
                tb0 = tau_sb[:, 0, hph, :].unsqueeze(1).broadcast_to([128, 8, 256])
                vb1 = v_sb[:, 1, hph, :].unsqueeze(1).broadcast_to([128, 8, 256])
                tb1 = tau_sb[:, 1, hph, :].unsqueeze(1).broadcast_to([128, 8, 256])
                # pod0, in place: q <- t + min(-min(t,tau), tau) = t - clip(t)
                nc.vector.tensor_tensor(t[:], qs, vb0, alu.mult)
                nc.vector.tensor_tensor(m[:], t[:], tb0, alu.min)
                nc.vector.scalar_tensor_tensor(m[:], m[:], -1.0, tb0, alu.mult, alu.min)
                nc.vector.tensor_tensor(qs, t[:], m[:], alu.add)
                # pod1, accumulate into q
                nc.vector.tensor_tensor(t[:], q2s, vb1, alu.mult)
                nc.vector.tensor_tensor(m[:], t[:], tb1, alu.min)
                nc.vector.scalar_tensor_tensor(m[:], m[:], -1.0, tb1, alu.mult, alu.min)
                nc.vector.tensor_tensor(t[:], t[:], m[:], alu.add)
                nc.vector.tensor_tensor(qs, qs, t[:], alu.add)

        # ---- inverse transform + int4-blk16 quantization, per plane ----
        for o in range(C):
            t2 = t2s[o % 2]
            for wph in range(2):
                ps = pps.tile([128, 256], dt.float32, tag="ps")
                nc.tensor.matmul(ps[:], q[:, 0, o, wph * 128:(wph + 1) * 128],
                                 hm_sb[:, 0, :], start=True, stop=False)
                nc.tensor.matmul(ps[:], q[:, 1, o, wph * 128:(wph + 1) * 128],
                                 hm_sb[:, 1, :], start=False, stop=True)
                nc.any.tensor_copy(t2[:, wph, :], ps[:])
            for hh in range(2):
                ps = pps.tile([128, 256], dt.float32, tag="ps")
                nc.tensor.matmul(ps[:], t2[:, 0, hh * 128:(hh + 1) * 128],
                                 hm_sb[:, 0, :], start=True, stop=False)
                nc.tensor.matmul(ps[:], t2[:, 1, hh * 128:(hh + 1) * 128],
                                 hm_sb[:, 1, :], start=False, stop=True)
                k = (2 * o + hh) % 2
                am, s8, sb, qt, pkt = ams[k], s8s[k], sbs[k], qts[k], pkts[k]
                cv = ps[:].rearrange("p (b e) -> p b e", b=16)
                nc.vector.tensor_reduce(am[:], cv, axis=mybir.AxisListType.X,
                                        op=alu.max, apply_absolute_value=True)
                nc.vector.tensor_scalar_mul(am[:], am[:], QSCALE)
                nc.vector.tensor_scalar_max(am[:], am[:], SMIN)
                nc.any.tensor_copy(s8[:], am[:])     # fp32 -> fp8 (wire scale)
                nc.any.tensor_copy(sb[:], s8[:])     # fp8 -> fp32 (consistent)
                nc.vector.reciprocal(sb[:], sb[:])
                sb_b = sb[:].unsqueeze(2).broadcast_to([128, 16, 16])
                nc.vector.tensor_tensor(qt[:], cv, sb_b, alu.mult)
                nc.vector.tensor_scalar_add(qt[:], qt[:], 8.0 + MAGIC)
                nc.vector.tensor_scalar_add(qt[:], qt[:], -MAGIC)
                nc.vector.tensor_scalar_min(qt[:], qt[:], 15.0)
                qv = qt[:].rearrange("p b e -> p (b e)").rearrange(
                    "p (k two) -> p k two", two=2)
                nc.vector.scalar_tensor_tensor(pkt[:], qv[:, :, 0], 16.0,
                                               qv[:, :, 1], alu.mult, alu.add)
                nc.gpsimd.dma_start(pks[o // (C // 8)][o % (C // 8), hh], pkt[:])
                nc.gpsimd.dma_start(s8_out[o, hh], s8[:])

    nc.compile()
    nc.finalize()
    return nc


def _prep_consts(v, conv_w, tau):
    hm = _haar_matrix(H)
    hmt = np.ascontiguousarray(hm.T)
    hmt_s = np.ascontiguousarray(hmt.reshape(2, 128, 256).transpose(1, 0, 2)).astype(BF16)
    hm_s = np.ascontiguousarray(hm.reshape(2, 128, 256).transpose(1, 0, 2)).astype(BF16)
    wdup = np.ascontiguousarray(conv_w.transpose(2, 0, 1)).astype(BF16)  # [c, pod, o]
    v_s = np.ascontiguousarray(
        v.reshape(P, 2, 128, 256).transpose(2, 0, 1, 3)).astype(BF16)
    tau_s = np.ascontiguousarray(
        tau.reshape(P, 2, 128, 256).transpose(2, 0, 1, 3)).astype(BF16)
    return {"hmt_s": hmt_s, "hm_s": hm_s, "wdup": wdup, "v_s": v_s, "tau_s": tau_s}


def _build_fast(nc, mesh):
    """jit(shard_map) executing the bass NEFF on 8 cores (bass2jax custom
    call), with device-resident zero output buffers."""
    import jax
    from jax.sharding import PartitionSpec, NamedSharding
    try:
        from jax.experimental.shard_map import shard_map
    except ImportError:
        from jax.shard_map import shard_map
    from concourse import bass2jax, mybir

    bass2jax.install_neuronx_cc_hook()

    pname = nc.partition_id_tensor.name if nc.partition_id_tensor else None
    in_names, out_names, out_avals = [], [], []
    for alloc in nc.m.functions[0].allocations:
        if not isinstance(alloc, mybir.MemoryLocationSet):
            continue
        name = alloc.memorylocations[0].name
        if alloc.kind == "ExternalInput":
            if name != pname:
                in_names.append(name)
        elif alloc.kind == "ExternalOutput":
            out_names.append(name)
            out_avals.append(jax.core.ShapedArray(
                tuple(alloc.tensor_shape), mybir.dt.np(alloc.dtype)))

    bind_names = list(in_names) + list(out_names)
    if pname is not None:
        bind_names.append(pname)

    def _body(*args):
        operands = list(args)
        if pname is not None:
            operands.append(bass2jax.partition_id_tensor())
        outs = bass2jax._bass_exec_p.bind(
            *operands,
            out_avals=tuple(out_avals),
            in_names=tuple(bind_names),
            out_names=tuple(out_names),
            lowering_input_output_aliases=(),
            sim_require_finite=True,
            sim_require_nnan=True,
            nc=nc,
        )
        return tuple(outs)

    n_args = len(in_names) + len(out_avals)
    fast = jax.jit(shard_map(
        _body, mesh=mesh,
        in_specs=(PartitionSpec("core"),) * n_args,
        out_specs=(PartitionSpec("core"),) * len(out_names),
        check_rep=False))
    sh = NamedSharding(mesh, PartitionSpec("core"))
    zeros_dev = [
        jax.device_put(
            np.zeros((NCORES * a.shape[0], *a.shape[1:]), a.dtype), sh)
        for a in out_avals
    ]
    for z in zeros_dev:
        z.block_until_ready()
    return fast, in_names, out_names, zeros_dev


def _make_luts():
    # packed byte -> (hi_code-8, lo_code-8) as adjacent fp32 in one f64 slot
    idx = np.arange(256)
    lut64 = np.empty(256, dtype="<f8")
    lv = lut64.view("<f4").reshape(256, 2)
    lv[:, 0] = (idx >> 4) - 8.0
    lv[:, 1] = (idx & 15) - 8.0
    # fp8 byte -> fp32 scale
    lut_fp8 = np.arange(256, dtype=np.uint8).view(FP8).astype(np.float32)
    return lut64, lut_fp8


def _ensure_built():
    import jax
    from jax.sharding import Mesh
    if "fast" in _state:
        return
    nc = _build_nc()
    mesh = Mesh(np.asarray(jax.devices()[:NCORES]), ("core",))
    _state["mesh"] = mesh
    _state["fast"] = _build_fast(nc, mesh)
    _state["lut64"], _state["lut_fp8"] = _make_luts()
    _state["xh"] = [[np.empty((NCORES * CSP, 2, 128, 160), dtype=np.uint8)
                     for _ in range(NSPLIT)] for _ in range(NCHUNK)]
    _state["sch"] = [[np.empty((NCORES * 128, CSP, 2), dtype=FP8)
                      for _ in range(NSPLIT)] for _ in range(NCHUNK)]
    _state["qtmp"] = np.empty((NCORES, CSP, 2, 128, 256), dtype=np.float32)
    _state["ctmp"] = np.empty((NCORES, CSP, 2, 128, 256), dtype=np.uint8)
    _state["ybuf"] = np.empty((B, C, H, W), dtype=np.float32)


def _ensure_consts(v, conv_w, tau):
    import jax
    from jax.sharding import PartitionSpec, NamedSharding
    key = (v.tobytes(), conv_w.tobytes(), tau.tobytes())
    cached = _state.get("consts_key")
    if cached is not None and cached == key:
        return
    consts = _prep_consts(v, conv_w, tau)
    sh = NamedSharding(_state["mesh"], PartitionSpec("core"))
    dev = {}
    for n, a in consts.items():
        g = np.broadcast_to(a[None], (NCORES,) + a.shape).reshape(
            (NCORES * a.shape[0],) + a.shape[1:])
        dev[n] = jax.device_put(np.ascontiguousarray(g), sh)
    for d in dev.values():
        d.block_until_ready()
    _state["consts_dev"] = dev
    _state["consts_key"] = key


def _pack_chunk_split(x, k, s):
    """6-bit rowscale pack of batches k*8.., channels s*CSP.. ->
    (packed [8*CSP,2,128,192] u8, scales [8*128,CSP,2] f32)."""
    pkb = _state["xh"][k][s]
    scb = _state["sch"][k][s]
    tmp = _state["qtmp"]
    cb = _state["ctmp"]
    src = x[k * NCORES:(k + 1) * NCORES, s * CSP:(s + 1) * CSP].reshape(
        NCORES, CSP, 2, 128, 256)
    am = np.maximum(src.max(axis=-1), -src.min(axis=-1))  # row absmax, no temp
    s8 = np.maximum(am * ISCALE, SMIN).astype(FP8)      # wire scales (fp8)
    s_eff = s8.astype(np.float32)                       # value device decodes
    np.multiply(src, (1.0 / s_eff)[..., None], out=tmp)
    tmp += 16.4995
    np.minimum(tmp, 31.9, out=tmp)   # fp8 scale can round down ~6% -> clip
    cb[...] = tmp                                       # trunc -> round
    cv = cb.reshape(NCORES, CSP, 2, 128, 32, 8)
    pv = pkb.reshape(NCORES, CSP, 2, 128, 32, 5)
    c = [cv[..., i] for i in range(8)]
    pv[..., 0] = (c[0] << 3) | (c[1] >> 2)
    pv[..., 1] = (c[1] << 6) | (c[2] << 1) | (c[3] >> 4)
    pv[..., 2] = (c[3] << 4) | (c[4] >> 1)
    pv[..., 3] = (c[4] << 7) | (c[5] << 2) | (c[6] >> 3)
    pv[..., 4] = (c[6] << 5) | c[7]
    scb[...] = s8.transpose(0, 3, 1, 2).reshape(NCORES * 128, CSP, 2)
    return pkb, scb


def _unpack_quarter(x, ybuf, k, qtr, pk_np, s8_np):
    """Reconstruct y for chunk k, plane eighth `qtr` (8 channels)."""
    lut64, lut_fp8 = _state["lut64"], _state["lut_fp8"]
    cq = C // 8
    c64 = lut64[pk_np.reshape(-1)]
    codes = c64.view("<f4").reshape(NCORES, cq, 2, 128, 16, 16)
    scales = lut_fp8[s8_np.reshape(NCORES, C, 2, 128, 16).view(np.uint8)
                     [:, qtr * cq:(qtr + 1) * cq]]
    np.multiply(codes, scales[..., None], out=codes)
    o0 = qtr * cq
    xs = x[k * NCORES:(k + 1) * NCORES, o0:o0 + cq].reshape(
        NCORES, cq, 2, 128, 256)
    ysl = ybuf[k * NCORES:(k + 1) * NCORES, o0:o0 + cq].reshape(
        NCORES, cq, 2, 128, 256)
    np.add(xs, codes.reshape(NCORES, cq, 2, 128, 256), out=ysl)


def kernel(x, v, conv_w, tau):
    # retries with a full rebuild: a transient relay/device error mid-call
    # would otherwise fail the whole run (NEFF compile is disk-cached, so a
    # rebuild costs seconds, not minutes)
    for attempt in range(3):
        try:
            return _kernel(x, v, conv_w, tau)
        except Exception:
            _state.clear()
            if attempt == 2:
                raise
    return None  # unreachable


def _kernel(x, v, conv_w, tau):
    import jax
    from jax.sharding import PartitionSpec, NamedSharding

    x = np.asarray(x, dtype=np.float32)
    with _lock:
        _ensure_built()
        _ensure_consts(np.asarray(v, np.float32),
                       np.asarray(conv_w, np.float32),
                       np.asarray(tau, np.float32))
        fast, in_names, out_names, zeros_dev = _state["fast"]
        sh = NamedSharding(_state["mesh"], PartitionSpec("core"))
        consts_dev = _state["consts_dev"]

        i_pk = [out_names.index(f"pk{h}") for h in range(8)]
        i_s8 = out_names.index("s8")
        fetch_order = [i_s8] + i_pk

        # pack + async upload per chunk; dispatch exec as soon as its
        # inputs are queued so downloads start during chunk1 uploads
        dev_in = [[None] * NSPLIT for _ in range(NCHUNK)]
        dev_sc = [[None] * NSPLIT for _ in range(NCHUNK)]
        outs = []
        for k in range(NCHUNK):
            for s in range(NSPLIT):
                pkb, scb = _pack_chunk_split(x, k, s)
                dev_sc[k][s] = jax.device_put(scb, sh)
                dev_in[k][s] = jax.device_put(pkb, sh)
            args = []
            for n in in_names:
                if n.startswith("xp"):
                    args.append(dev_in[k][int(n[2:])])
                elif n.startswith("sc"):
                    args.append(dev_sc[k][int(n[2:])])
                else:
                    args.append(consts_dev[n])
            outs.append(fast(*args, *zeros_dev))
            for i in fetch_order:
                try:
                    outs[k][i].copy_to_host_async()
                except Exception:
                    pass

        ybuf = _state["ybuf"]
        for k in range(NCHUNK):
            s8_np = np.asarray(outs[k][i_s8])
            for qtr in range(8):
                pk_np = np.asarray(outs[k][i_pk[qtr]])
                _unpack_quarter(x, ybuf, k, qtr, pk_np, s8_np)
    return ybuf
